# revision 72
# baseline (speedup 1.0000x reference)
"""Trainium2 Bass kernel for nn_DecoderLayer (dense transformer decoder layer).

Sharding: data-parallel over batch (16 batches -> 8 cores x 2 each). Each core
runs the full decoder layer on its batch slice; no collectives.

v17 (from the v7 baseline, 1252us -> 926us):
- fp8e4m3 DoubleRow matmuls for every projection (SA/CA Q,K,V,O): weights
  x1024 and activations x16 host-quantized (or on-chip via scaled
  evictions), each DR matmul contracting 256 features, ~1.67x the bf16
  rate. The scale products fold into the softmax exp scale, a scaled
  ones-column (softmax denominator), and fused (psum*s + residual) DVE
  ops -- zero extra instructions. FFN stays bf16 (fp8 there costs ~1.7e-2
  rel err, over the budget; attention-path fp8 is invisible because
  attention outputs are near-uniform averages).
- scores row-tiling: the two heads of a pair sit at PE base partitions
  0/64 (K=64 each); interleaving their matmuls runs them concurrently on
  disjoint row-groups (second matmul of a pair measures ~4ns). Softmax
  normalize broadcasts are similarly paired on disjoint tile groups.
  Keeping scores/PV operands bf16 matters: fp8 stationaries lose FWL and
  cost ~35% on these small matmuls.
- phase overlap: CA K/V weights + enc chunk 0 are loaded and projected
  inside the SA interleave slots; LN1/LN2 and the CA Q-projection are
  per-batch so each batch's work starts as soon as its residual is ready;
  LN2 runs inside the chunk loop; the first fc1 weight tiles prefetch
  during the chunk loop; LN3/transpose/output-DMA pipeline per tcx.

Layout: activations are feature-major ("xT": [feature partitions, token
free]) so every linear is psum = W^T.T @ xT with bf16 operands and fp32 PSUM
accumulation. Attention uses transposed scores S^T = K_hT.T @ Q_hT
([j partitions, i free]), exp without max-subtraction (scores are bounded),
and a ones-column appended to V so the softmax denominator comes out of the
same PV matmul. LayerNorm runs feature-major with partition sums via
ones-vector matmuls and per-token broadcast via K=1 matmuls.

The cross-attention main loop is software-pipelined: chunk c's K/V-projection
matmul groups interleave with chunk c-1's attention at head-pair granularity
(scores and PV split one slot apart), so the PE always has projection work
while ACT grinds the exps and HAM stays at full clock.
"""
import sys
import numpy as np

sys.path.insert(0, '/opt/trn_rl_repo')

import ml_dtypes  # noqa: E402
import concourse.bass as bass  # noqa: E402
import concourse.tile as tile  # noqa: E402
from concourse import bacc, mybir  # noqa: E402
from concourse.bass_utils import run_bass_kernel_spmd  # noqa: E402
from concourse.masks import make_identity  # noqa: E402
from contextlib import ExitStack  # noqa: E402

F32 = mybir.dt.float32
BF16 = mybir.dt.bfloat16
FP8 = mybir.dt.float8e4
AF = mybir.ActivationFunctionType
BF16_NP = ml_dtypes.bfloat16
FP8_NP = ml_dtypes.float8_e4m3
DR = mybir.MatmulPerfMode.DoubleRow

EPS = 1e-5
N_CORES = 8
# fp8 scaling: enc_mem x MSCALE, ca K/V weights x WSCALE; product folds out
# via the exp scale (K path) and a scaled ones-column (V path).
MSCALE = 16.0
WSCALE = 1024.0
KVSCALE = MSCALE * WSCALE
# attention outputs are quantized x64 (folded into the softmax-normalize
# broadcast), so out-proj psums carry 64*1024:
OSCALE = 64.0 * WSCALE


def build_decoder(nc, tc, ctx, B_loc, NQ, S, W, NH, MLP, JC=512, suffix=""):
    HD = W // NH
    assert HD == 64 and NQ % 128 == 0 and W % 512 == 0 and JC % 128 == 0
    T = B_loc * NQ          # decoder tokens per core
    TC = T // 128
    WC = W // 128
    MC = MLP // 128
    NJC = S // JC           # enc chunks per batch
    JSC = JC // 128
    NQC = NQ // 128
    SCALE = float(W) ** -0.5
    HPC = 128 // HD         # heads per feature chunk (2)

    # Transformed inputs (host-side marshalling):
    #   *_wt:  [128, O/128, I/128, 128] bf16, wt[p,ob,c,n] = W[ob*128+n, c*128+p]
    #   qT/qkT/peT: [128, WC, T] bf16 feature-major (qkT = q + pos_enc)
    #   mT:    [128, WC, T_enc] bf16 feature-major (T_enc = B_loc*S)
    #   *_col: [128, n/128] f32 per-partition param columns
    dram = {}
    for name, shape, dt in (
        [('qT', [128, WC, T], BF16), ('qTq', [128, WC, T], FP8),
         ('qkTq', [128, WC, T], FP8),
         ('peT16', [128, WC, T], BF16), ('mT', [128, WC, B_loc * S], FP8),
         ('sa_wqt_dr', [128, WC, WC // 2, 2, 128], FP8),
         ('sa_wkt_dr', [128, WC, WC // 2, 2, 128], FP8),
         ('sa_wvt_mov', [128, WC // 2, 2, W], FP8),
         ('ca_wqt_dr', [128, WC, WC // 2, 2, 128], FP8),
         ('ca_wkt_dr', [128, WC, WC // 2, 2, 128], FP8),
         ('ca_wvt_mov', [128, WC // 2, 2, W], FP8),
         ('sa_wot_dr', [128, WC, WC // 2, 2, 128], FP8),
         ('ca_wot_dr', [128, WC, WC // 2, 2, 128], FP8),
         ('ffn_w1t', [128, MC, WC, 128], BF16),
         ('ffn_w2t', [128, WC, MC, 128], BF16),
         ('b1_col', [128, MC], F32), ('b2_col', [128, WC], F32)] +
        [(f'ln{i}_{gb}_col', [128, WC], F32)
         for i in (1, 2, 3) for gb in ('g', 'b')]
    ):
        if suffix:
            dram[name] = build_decoder._dram_cache[name]
        else:
            dram[name] = nc.dram_tensor(name, shape, dt, kind="ExternalInput")
    build_decoder._dram_cache = dict(dram)
    out_d = nc.dram_tensor("out" + suffix, [B_loc, NQ, W], F32,
                           kind="ExternalOutput")
    out_flat = out_d.rearrange("b n w -> (b n) w")

    # ---------------- global pools ----------------
    consts = ctx.enter_context(tc.tile_pool(name="consts", bufs=1))
    persist = ctx.enter_context(tc.tile_pool(name="persist", bufs=1))
    scratch = ctx.enter_context(tc.tile_pool(name="scratch", bufs=2))
    # pools that span the SA and CA phases (CA K/V weights + enc chunks are
    # loaded and projected during SA to fill its pipeline bubbles); freed
    # before the FFN so its pool fits.
    w1_pool = ctx.enter_context(tc.tile_pool(name="w1_pool", bufs=1))
    att_ctx = ExitStack()
    ca_w = att_ctx.enter_context(tc.tile_pool(name="ca_w", bufs=1))
    cjc = att_ctx.enter_context(tc.tile_pool(name="ca_jc", bufs=1))
    mm_ps = ctx.enter_context(tc.tile_pool(name="mm_ps", bufs=2, space="PSUM"))
    sc_ps = ctx.enter_context(tc.tile_pool(name="sc_ps", bufs=4, space="PSUM"))
    pv_ps = ctx.enter_context(tc.tile_pool(name="pv_ps", bufs=2, space="PSUM"))

    ident = consts.tile([128, 128], F32, tag="ident")
    make_identity(nc, ident[:])
    ones_f = consts.tile([128, 128], F32, tag="ones_f")
    nc.gpsimd.memset(ones_f[:], 1.0)
    ones_b = consts.tile([128, 128], BF16, tag="ones_b")
    nc.vector.tensor_copy(ones_b[:], ones_f[:])
    ones_v = consts.tile([128, 128], F32, tag="ones_v")
    nc.gpsimd.memset(ones_v[:], KVSCALE)
    c64_f = consts.tile([33, 128], F32, tag="c64_f")
    nc.gpsimd.memset(c64_f[:], 64.0)
    c64_b = consts.tile([33, 128], BF16, tag="c64_b")
    nc.vector.tensor_copy(c64_b[:], c64_f[:])
    eps_t = consts.tile([1, 1], F32, tag="eps")
    nc.gpsimd.memset(eps_t[:], EPS)

    cols = {}
    for name in ['ln1_g', 'ln1_b', 'ln2_g', 'ln2_b', 'ln3_g', 'ln3_b',
                 'b2']:
        cn = name + '_col' if name != 'b2' else 'b2_col'
        t = consts.tile([128, WC], F32, tag=cn, name=cn)
        nc.sync.dma_start(t[:], dram[cn][:, :])
        cols[name] = t
    b1_col = consts.tile([128, MC], F32, tag="b1c", name="b1_col")
    nc.sync.dma_start(b1_col[:], dram['b1_col'][:, :])

    # ---------------- helpers ----------------
    def load_wT(pool, tag, wt_d, name, bufs=1):
        """Weight tile from pre-transposed DRAM; per-ob DMAs so the first
        GEMM group can start after one transfer."""
        OB = wt_d.shape[1]
        wt = pool.tile([128] + list(wt_d.shape[1:]), wt_d.dtype, tag=tag,
                       name=name, bufs=bufs)
        for ob in range(OB):
            nc.sync.dma_start(wt[:, ob], wt_d[:, ob])
        return wt

    def gemm(psum, wt, ob, rhs, ICn):
        """psum[oc 128, N] += sum_ic wt[:, ob, ic, :].T @ rhs(ic)."""
        for ic in range(ICn):
            nc.tensor.matmul(psum, wt[:, ob, ic, :], rhs(ic),
                             start=(ic == 0), stop=(ic == ICn - 1))

    def gemm_dr(psum, wt_dr, ob, rhs_pair, Gn):
        """fp8 DoubleRow: psum += sum_g wt_dr[:, ob, g].T @ rhs_pair(g),
        each g contracting a 256-feature pair-block."""
        for g in range(Gn):
            nc.tensor.matmul(psum, wt_dr[:, ob, g], rhs_pair(g),
                             start=(g == 0), stop=(g == Gn - 1),
                             perf_mode=DR)

    # --- CA enc-chunk load + K/V projection (wkt2/wvt2 assigned in the SA
    # scope before first call; chunk 0 is projected during SA) ---
    def load_chunk(c):
        b, jc = c // NJC, c % NJC
        mT = cjc.tile([128, WC, JC], FP8, tag="mT", bufs=2, name="mT")
        nc.sync.dma_start(
            mT[:], dram['mT'][:, :, b * S + jc * JC:b * S + (jc + 1) * JC])
        k2T = cjc.tile([128, WC, JC], BF16, tag="k2T", bufs=2, name="k2T")
        vext = cjc.tile([128, JSC, NH, HD + 1], BF16, tag="vext",
                        bufs=2, name="vext_ca")
        return mT, k2T, vext

    def proj_closures(mT, k2T, vext):
        cls = []
        for ob in range(WC):
            def kproj(ob=ob, mT=mT, k2T=k2T):
                ps = mm_ps.tile([128, JC], F32, tag="mm", name="ps_k2")
                for g in range(WC // 2):
                    nc.tensor.matmul(
                        ps[:, 0:JC], wkt2[:, ob, g],
                        mT[:, 2 * g:2 * g + 2, :],
                        start=(g == 0), stop=(g == WC // 2 - 1),
                        perf_mode=DR)
                nc.vector.tensor_copy(k2T[:, ob, :], ps[:, 0:JC])
            cls.append(kproj)
        for sj in range(JSC):
            for oh in range(W // 512):
                def vproj(sj=sj, oh=oh, mT=mT, vext=vext):
                    ps = mm_ps.tile([128, 512], F32, tag="mm",
                                    name="ps_v2")
                    for g in range(WC // 2):
                        nc.tensor.matmul(
                            ps[:, 0:512],
                            mT[:, 2 * g:2 * g + 2,
                               sj * 128:(sj + 1) * 128],
                            wvt2[:, g, :, oh * 512:(oh + 1) * 512],
                            start=(g == 0),
                            stop=(g == WC // 2 - 1), perf_mode=DR)
                    nh0 = oh * (512 // HD)
                    nc.vector.tensor_copy(
                        vext[:, sj, nh0:nh0 + 512 // HD, 0:HD],
                        ps[:, 0:512].rearrange("p (h d) -> p h d", d=HD))
                    if oh == W // 512 - 1:
                        nc.vector.tensor_copy(vext[:, sj, :, HD],
                                              ones_v[:, 0:NH])
                cls.append(vproj)
        return cls

    def layernorm(x_fn, n_chunks, N, g_col, b_col, out_fn,
                  out2_fn=None, g2_col=None, b2_col=None):
        """Feature-major LN over the partition (feature) dim. Optional
        second output (out2_fn) applies (g2, b2) — used to emit a scaled
        fp8 copy alongside the bf16 one."""
        ps_s = sc_ps.tile([1, N], F32, tag="sc", name="ps_s",
                          padded_shape=[1, 512])
        for ic in range(n_chunks):
            nc.tensor.matmul(ps_s[0:1, :], ones_b[:, 0:1], x_fn(ic),
                             start=(ic == 0), stop=(ic == n_chunks - 1))
        ps_q = sc_ps.tile([1, N], F32, tag="sc", name="ps_q",
                          padded_shape=[1, 512])
        for ic in range(n_chunks):
            sq = scratch.tile([128, N], BF16, tag="sq", name="sq",
                              padded_shape=[128, 512])
            nc.vector.tensor_mul(sq[:, 0:N], x_fn(ic), x_fn(ic))
            nc.tensor.matmul(ps_q[0:1, :], ones_b[:, 0:1], sq[:, 0:N],
                             start=(ic == 0), stop=(ic == n_chunks - 1))
        inv_w = 1.0 / (n_chunks * 128)
        mu = scratch.tile([1, N], BF16, tag="st_mu", bufs=1, name="mu")
        nc.scalar.activation(mu[0:1, :], ps_s[0:1, :], AF.Copy, scale=inv_w)
        ex2 = scratch.tile([1, N], F32, tag="st_e", bufs=1, name="ex2")
        nc.scalar.activation(ex2[0:1, :], ps_q[0:1, :], AF.Copy, scale=inv_w)
        mu2 = scratch.tile([1, N], F32, tag="st_x", bufs=1, name="mu2")
        nc.vector.tensor_mul(mu2[0:1, :], mu[0:1, :], mu[0:1, :])
        var = scratch.tile([1, N], F32, tag="st_v", bufs=1, name="var")
        nc.vector.tensor_sub(var[0:1, :], ex2[0:1, :], mu2[0:1, :])
        sd = scratch.tile([1, N], F32, tag="st_x", bufs=1, name="sd")
        nc.scalar.activation(sd[0:1, :], var[0:1, :], AF.Sqrt,
                             bias=eps_t[0:1, 0:1])
        rstd = scratch.tile([1, N], BF16, tag="st_r", bufs=1, name="rstd")
        nc.vector.reciprocal(rstd[0:1, :], sd[0:1, :])
        ps_mu = sc_ps.tile([128, N], F32, tag="sc", name="ps_mu",
                           padded_shape=[128, 512])
        nc.tensor.matmul(ps_mu[:, 0:N], ones_b[0:1, :], mu[0:1, :])
        ps_rs = sc_ps.tile([128, N], F32, tag="sc", name="ps_rs",
                           padded_shape=[128, 512])
        nc.tensor.matmul(ps_rs[:, 0:N], ones_b[0:1, :], rstd[0:1, :])
        for ic in range(n_chunks):
            xm = scratch.tile([128, N], F32, tag="xm", bufs=1, name="xm",
                              padded_shape=[128, 512])
            nc.vector.tensor_sub(xm[:, 0:N], x_fn(ic), ps_mu[:, 0:N])
            nc.vector.tensor_mul(xm[:, 0:N], xm[:, 0:N], ps_rs[:, 0:N])
            nc.scalar.activation(out_fn(ic), xm[:, 0:N], AF.Identity,
                                 bias=b_col[:, ic:ic + 1],
                                 scale=g_col[:, ic:ic + 1])
            if out2_fn is not None:
                nc.scalar.activation(out2_fn(ic), xm[:, 0:N], AF.Identity,
                                     bias=b2_col[:, ic:ic + 1],
                                     scale=g2_col[:, ic:ic + 1])

    def normalize_pair(hp, src0, src1, oT, col):
        """oT head-pair slice = 64 * src[0:HD] / src[HD] (softmax sums
        row; the x64 feeds the fp8 out-proj, descaled after the matmul).
        The two heads' reciprocal broadcasts run on disjoint PE tile
        groups (rows 0-31 cols 0-63 vs rows 32-63 cols 64-127) so they
        execute concurrently."""
        fc = hp
        rec = scratch.tile([33, NQ], BF16, tag="rec", bufs=2, name="rec")
        nc.vector.reciprocal(rec[0:1, :], src0[HD:HD + 1, :])
        nc.vector.reciprocal(rec[32:33, :], src1[HD:HD + 1, :])
        ps_b = mm_ps.tile([128, NQ], F32, tag="mm", name="ps_bc")
        nc.tensor.matmul(ps_b[0:HD, :], c64_b[0:1, 0:HD], rec[0:1, :])
        nc.tensor.matmul(ps_b[HD:128, :], c64_b[32:33, 0:HD],
                         rec[32:33, :], tile_position=(32, 64))
        nc.vector.tensor_mul(oT[0:HD, fc, col:col + NQ], src0[0:HD, :],
                             ps_b[0:HD, :])
        nc.vector.tensor_mul(oT[HD:128, fc, col:col + NQ], src1[0:HD, :],
                             ps_b[HD:128, :])

    def sa_scores_pair(b, ksaT, qsaT, hp):
        """Scores + exp for head pair hp of batch b (SA, NQC key blocks).

        The two heads of the pair sit at base partitions 0/64 (K=64 each),
        so their matmuls land on disjoint PE row-groups; interleaving them
        lets the array run both concurrently (row tiling)."""
        fc = hp
        es, pss = [], []
        for sub in range(2):
            es.append(sa_pool[0].tile([128, NQC, NQ], FP8, tag="expsa",
                                      bufs=16, name="esa"))
            pss.append(sc_ps.tile([128, 2, NQ], F32, tag="sc",
                                  name="ps_sc"))
        for js in range(NQC):
            for sub in range(2):
                off = sub * HD
                nc.tensor.matmul(
                    pss[sub][:, js, :],
                    ksaT[off:off + HD, fc, b * NQ + js * 128:
                         b * NQ + (js + 1) * 128],
                    qsaT[off:off + HD, fc, b * NQ:(b + 1) * NQ])
        for sub in range(2):
            nc.scalar.activation(es[sub][:, :, :], pss[sub][:, 0:NQC, :],
                                 AF.Exp, scale=SCALE / MSCALE ** 2)
        return es

    def sa_pv_pair(b, vext_all, hp, es, oT):
        ps_o = pv_ps.tile([HD + 1, 2, NQ], F32, tag="pv", name="ps_pv2")
        for sub, e in ((0, es[0]), (1, es[1])):
            h = 2 * hp + sub
            for js in range(NQC):
                nc.tensor.matmul(
                    ps_o[0:HD + 1, sub, :],
                    vext_all[:, b * NQC + js, h, :], e[:, js, :],
                    start=(js == 0), stop=(js == NQC - 1))
        pv_sb = sa_pool[0].tile([HD + 1, 2, NQ], F32, tag="pvsb", bufs=2,
                                name="pv_sb")
        nc.vector.tensor_copy(pv_sb[:], ps_o[:])
        normalize_pair(hp, pv_sb[:, 0, :], pv_sb[:, 1, :], oT, b * NQ)

    # ================= P0: self-attention =================
    sa_pool = [None]
    x1T = persist.tile([128, WC, T], BF16, tag="x1T", name="x1T")
    qT = persist.tile([128, WC, T], BF16, tag="qT", name="qT")
    peT16 = persist.tile([128, WC, T], BF16, tag="peT", name="peT16")
    with nc.named_scope("sa"), \
         tc.tile_pool(name="sa_w", bufs=1) as sa_w, \
         tc.tile_pool(name="sa", bufs=1) as sa:
        sa_pool[0] = sa
        qkTq = sa.tile([128, WC, T], FP8, tag="qkTq", name="qkTq")
        # startup order: first Q-GEMM needs wqt[ob0] + qkTq, so those DMAs
        # go first (split across queues); the rest trail behind.
        wqt = load_wT(sa_w, "wtA", dram['sa_wqt_dr'], "sa_wq_dr", bufs=2)
        for ic in range(WC // 2):
            nc.sync.dma_start(qkTq[:, 2 * ic:2 * ic + 2, :],
                              dram['qkTq'][:, 2 * ic:2 * ic + 2, :])
        qTq = sa.tile([128, WC, T], FP8, tag="qTq", name="qTq")
        nc.sync.dma_start(qTq[:], dram['qTq'][:, :, :])
        nc.sync.dma_start(qT[:], dram['qT'][:, :, :])
        nc.sync.dma_start(peT16[:], dram['peT16'][:, :, :])
        qsaT = sa.tile([128, WC, T], FP8, tag="big", bufs=2, name="qsaT")
        for ob in range(WC):
            ps = mm_ps.tile([128, T], F32, tag="mm", name="ps_q")
            gemm_dr(ps[:, 0:T], wqt, ob,
                    lambda g: qkTq[:, 2 * g:2 * g + 2, :], WC // 2)
            nc.scalar.activation(qsaT[:, ob, :], ps[:, 0:T], AF.Identity,
                                 scale=MSCALE / KVSCALE)
        wkt = load_wT(sa_w, "wtA", dram['sa_wkt_dr'], "sa_wk_dr", bufs=2)
        ksaT = sa.tile([128, WC, T], FP8, tag="big", bufs=2, name="ksaT")
        for ob in range(WC):
            ps = mm_ps.tile([128, T], F32, tag="mm", name="ps_k")
            gemm_dr(ps[:, 0:T], wkt, ob,
                    lambda g: qkTq[:, 2 * g:2 * g + 2, :], WC // 2)
            nc.scalar.activation(ksaT[:, ob, :], ps[:, 0:T], AF.Identity,
                                 scale=MSCALE / KVSCALE)
        wvt = sa_w.tile([128, WC // 2, 2, W], FP8, tag="wtA", bufs=2,
                        name="sa_wv_mov")
        for g in range(WC // 2):
            nc.sync.dma_start(wvt[:, g], dram['sa_wvt_mov'][:, g])
        wot = load_wT(sa_w, "wtA", dram['sa_wot_dr'], "sa_wo_dr", bufs=2)
        # CA K/V weights + enc chunk 0: loaded now, projected inside the
        # SA interleave (pure enc_mem work, no SA dependencies) to keep
        # the PE fed through SA's softmax/normalize dependency stalls.
        wvt2 = ca_w.tile([128, WC // 2, 2, W], FP8, tag="wtV",
                         name="ca_wv_mov")
        for g in range(WC // 2):
            nc.sync.dma_start(wvt2[:, g], dram['ca_wvt_mov'][:, g])
        wkt2 = ca_w.tile([128, WC, WC // 2, 2, 128], FP8, tag="wtK",
                         name="ca_wk_dr")
        for ob in range(WC):
            nc.sync.dma_start(wkt2[:, ob], dram['ca_wkt_dr'][:, ob])
        wqt2 = load_wT(ca_w, "wtQ", dram['ca_wqt_dr'], "ca_wq_dr")
        chunk0 = load_chunk(0)
        projs0 = proj_closures(*chunk0)
        vext_all = sa.tile([128, TC, NH, HD + 1], BF16, tag="vext",
                           name="vext_sa")

        def vproj_sa(tcx, oh):
            ps = mm_ps.tile([128, 512], F32, tag="mm", name="ps_v")
            for g in range(WC // 2):
                nc.tensor.matmul(
                    ps[:, 0:512],
                    qTq[:, 2 * g:2 * g + 2, tcx * 128:(tcx + 1) * 128],
                    wvt[:, g, :, oh * 512:(oh + 1) * 512],
                    start=(g == 0), stop=(g == WC // 2 - 1), perf_mode=DR)
            nh0 = oh * (512 // HD)
            nc.vector.tensor_copy(
                vext_all[:, tcx, nh0:nh0 + 512 // HD, 0:HD],
                ps[:, 0:512].rearrange("p (h d) -> p h d", d=HD))
            if oh == W // 512 - 1:
                nc.vector.tensor_copy(vext_all[:, tcx, :, HD],
                                      ones_v[:, 0:NH])

        osaT = sa.tile([128, WC, T], FP8, tag="osaT", name="osaT")
        x1pre = sa.tile([128, WC, T], BF16, tag="x1pre", name="x1pre")

        def oproj_sa(b, ob):
            ps = mm_ps.tile([128, NQ], F32, tag="mm", name="ps_o")
            gemm_dr(ps[:, 0:NQ], wot, ob,
                    lambda g: osaT[:, 2 * g:2 * g + 2, b * NQ:(b + 1) * NQ],
                    WC // 2)
            nc.vector.scalar_tensor_tensor(
                x1pre[:, ob, b * NQ:(b + 1) * NQ], ps[:, 0:NQ],
                1.0 / OSCALE, qT[:, ob, b * NQ:(b + 1) * NQ],
                mybir.AluOpType.mult, mybir.AluOpType.add)

        # Interleaved SA attention: V-projection groups hide exp(b0);
        # scores(b1) hide exp while PV(b0) drains; out-proj(b0) groups hide
        # exp while PV(b1) drains. Chunk-0 CA K/V projection groups are
        # woven through the later slots to fill dependency stalls.
        es0 = {}
        for hp in range(NH // 2):
            vproj_sa(hp % TC, hp // TC)
            es0[hp] = sa_scores_pair(0, ksaT, qsaT, hp)
            if hp >= 4:
                projs0[hp - 4]()
        es1 = {}
        for hp in range(NH // 2):
            sa_pv_pair(0, vext_all, hp, es0[hp], osaT)
            es1[hp] = sa_scores_pair(1, ksaT, qsaT, hp)
            projs0[4 + hp]()
        for hp in range(NH // 2):
            sa_pv_pair(1, vext_all, hp, es1[hp], osaT)
            oproj_sa(0, hp)
            if hp >= 4:
                projs0[8 + hp]()
        layernorm(lambda ic: x1pre[:, ic, 0:NQ], WC, NQ,
                  cols['ln1_g'], cols['ln1_b'],
                  lambda ic: x1T[:, ic, 0:NQ])
        # batch-0 CA queries: quantize now so the op clears the DVE queue
        # ahead of the SA-tail backlog (else the first CA Q-projection
        # stalls ~20us at the phase boundary).
        x1pq = ca_w.tile([128, WC, T], FP8, tag="x1pT", name="x1pq")
        nc.vector.scalar_tensor_tensor(
            x1pq[:, :, 0:NQ], x1T[:, :, 0:NQ], float(MSCALE),
            peT16[:, :, 0:NQ],
            mybir.AluOpType.mult, mybir.AluOpType.add)
        for ob in range(WC):
            oproj_sa(1, ob)
        layernorm(lambda ic: x1pre[:, ic, NQ:2 * NQ], WC, NQ,
                  cols['ln1_g'], cols['ln1_b'],
                  lambda ic: x1T[:, ic, NQ:2 * NQ])

    # ================= cross-attention =================
    with nc.named_scope("ca"), \
         tc.tile_pool(name="ca", bufs=1) as ca:
        q2T = ca.tile([128, WC, T], BF16, tag="q2T", name="q2T")
        with tc.tile_pool(name="ca_early", bufs=1) as cae:
            wot2 = load_wT(ca_w, "wtO", dram['ca_wot_dr'], "ca_wo_dr")
            # per-batch so chunk-0 scores (which need only batch 0's
            # queries) start as early as possible; b0's x1pq was already
            # quantized during the SA tail.
            for b in range(B_loc):
                sl = slice(b * NQ, (b + 1) * NQ)
                if b > 0:
                    nc.vector.scalar_tensor_tensor(
                        x1pq[:, :, sl], x1T[:, :, sl], float(MSCALE),
                        peT16[:, :, sl],
                        mybir.AluOpType.mult, mybir.AluOpType.add)
                for ob in range(WC):
                    ps = mm_ps.tile([128, NQ], F32, tag="mm", name="ps_q2")
                    gemm_dr(ps[:, 0:NQ], wqt2, ob,
                            lambda g: x1pq[:, 2 * g:2 * g + 2, sl],
                            WC // 2)
                    nc.vector.tensor_copy(q2T[:, ob, sl], ps[:, 0:NQ])

        ocaT = ca.tile([128, WC, T], FP8, tag="ocaT", name="ocaT")
        x2pre = ca.tile([128, WC, T], BF16, tag="x2pre", name="x2pre")
        x2T = persist.tile([128, WC, T], BF16, tag="x2T", name="x2T")

        def oproj_ca(b):
            """CA out-proj + residual + LN2 for one batch (issued right
            after its normalize, so it lands in the pipelined region, not
            the tail)."""
            for ob in range(WC):
                ps = mm_ps.tile([128, NQ], F32, tag="mm", name="ps_o2")
                gemm_dr(ps[:, 0:NQ], wot2, ob,
                        lambda g: ocaT[:, 2 * g:2 * g + 2,
                                       b * NQ:(b + 1) * NQ], WC // 2)
                nc.vector.scalar_tensor_tensor(
                    x2pre[:, ob, b * NQ:(b + 1) * NQ], ps[:, 0:NQ],
                    1.0 / OSCALE, x1T[:, ob, b * NQ:(b + 1) * NQ],
                    mybir.AluOpType.mult, mybir.AluOpType.add)
            layernorm(lambda ic: x2pre[:, ic, b * NQ:(b + 1) * NQ], WC, NQ,
                      cols['ln2_g'], cols['ln2_b'],
                      lambda ic: x2T[:, ic, b * NQ:(b + 1) * NQ])

        with tc.tile_pool(name="ca_acc", bufs=1) as cacc:
            # Software-pipelined chunk loop (see module docstring).
            acc = cacc.tile([HD + 1, NH, NQ], F32, tag="acc", name="acc_ca")
            n_chunks = B_loc * NJC

            def sc_closure(b, k2T, hp):
                """Scores for the head pair: interleave the two heads'
                K=64 matmuls (base partitions 0/64 -> disjoint PE
                row-groups) so the array runs both concurrently."""
                fc = hp
                es = [scratch.tile([128, JSC, NQ], FP8, tag="exp",
                                   bufs=6, name="e",
                                   padded_shape=[128, 4, NQ])
                      for _ in range(2)]
                for half in range(JSC // 2):
                    js0 = half * 2
                    pss = [sc_ps.tile([128, 2, NQ], F32, tag="sc",
                                      name="ps_sc") for _ in range(2)]
                    for s2 in range(2):
                        js = js0 + s2
                        for sub in range(2):
                            off = sub * HD
                            nc.tensor.matmul(
                                pss[sub][:, s2, :],
                                k2T[off:off + HD, fc,
                                    js * 128:(js + 1) * 128],
                                q2T[off:off + HD, fc, b * NQ:(b + 1) * NQ])
                    for sub in range(2):
                        nc.scalar.activation(es[sub][:, js0:js0 + 2, :],
                                             pss[sub][:, :, :], AF.Exp,
                                             scale=SCALE / KVSCALE ** 2)
                return es

            def pv_closure(vext, hp, es, first):
                ps_o = pv_ps.tile([HD + 1, 2, NQ], F32, tag="pv",
                                  name="ps_pv2")
                for sub, e in ((0, es[0]), (1, es[1])):
                    h = 2 * hp + sub
                    for js in range(JSC):
                        nc.tensor.matmul(ps_o[0:HD + 1, sub, :],
                                         vext[:, js, h, :], e[:, js, :],
                                         start=(js == 0),
                                         stop=(js == JSC - 1))
                if first:
                    nc.vector.tensor_copy(
                        acc[0:HD + 1, 2 * hp:2 * hp + 2, :],
                        ps_o[0:HD + 1, :, :])
                else:
                    nc.vector.tensor_add(
                        acc[0:HD + 1, 2 * hp:2 * hp + 2, :],
                        acc[0:HD + 1, 2 * hp:2 * hp + 2, :],
                        ps_o[0:HD + 1, :, :])

            # w1 prefetch: the first fc1 groups' weights stream in during
            # the chunk loop so the FFN starts without a DMA stall.
            w1_pre = []
            for i in range(4):
                t = w1_pool.tile([128, WC, 128], BF16, tag="w1t", bufs=4,
                                 name="w1t")
                nc.sync.dma_start(t[:], dram['ffn_w1t'][:, i])
                w1_pre.append(t)
            # chunk 0 was loaded + projected during SA; enter the loop with
            # it as `prev` so iteration 1 runs its attention.
            prev = (0, chunk0[1], chunk0[2], True, 0)
            cur = load_chunk(1)
            for c in range(1, n_chunks + 1):
                nxt = load_chunk(c + 1) if c + 1 < n_chunks else None
                projs = []
                if c < n_chunks:
                    mT, k2T, vext = cur
                    projs = proj_closures(mT, k2T, vext)
                # interleave: 2 proj groups, then scores(hp), then PV(hp-1)
                pend = None      # (hp, es) awaiting PV
                pi = 0
                for hp in range(NH // 2 + 1):
                    if pi < len(projs):
                        projs[pi]()
                        pi += 1
                    if hp < NH // 2 and prev is not None:
                        es = sc_closure(prev[0], prev[1], hp)
                    else:
                        es = None
                    if pi < len(projs):
                        projs[pi]()
                        pi += 1
                    if pend is not None:
                        pv_closure(prev[2], pend[0], pend[1], prev[3])
                    pend = (hp, es) if es is not None else None
                while pi < len(projs):
                    projs[pi]()
                    pi += 1
                if prev is not None:
                    if prev[4] == NJC - 1:   # last chunk of its batch
                        b_done = prev[0]
                        for hp2 in range(NH // 2):
                            normalize_pair(hp2, acc[:, 2 * hp2, :],
                                           acc[:, 2 * hp2 + 1, :], ocaT,
                                           b_done * NQ)
                        oproj_ca(b_done)
                if c < n_chunks:
                    prev = (c // NJC, k2T, vext, (c % NJC) == 0, c % NJC)
                    cur = nxt



    # ================= FFN =================
    att_ctx.close()
    with nc.named_scope("ffn"), tc.tile_pool(name="ffn", bufs=1) as ffn:
        hT = ffn.tile([128, MC, T], BF16, tag="hT", name="hT")
        for oc in range(MC):
            w1t = (w1_pre[oc] if oc < len(w1_pre) else
                   w1_pool.tile([128, WC, 128], BF16, tag="w1t", bufs=4,
                                name="w1t"))
            if oc >= len(w1_pre):
                nc.sync.dma_start(w1t[:], dram['ffn_w1t'][:, oc])
            ps = mm_ps.tile([128, T], F32, tag="mm", name="ps_h")
            for ic in range(WC):
                nc.tensor.matmul(ps[:, 0:T], w1t[:, ic, :], x2T[:, ic, :],
                                 start=(ic == 0), stop=(ic == WC - 1))
            nc.scalar.activation(hT[:, oc, :], ps[:, 0:T], AF.Relu,
                                 bias=b1_col[:, oc:oc + 1])
        x3pre = ffn.tile([128, WC, T], BF16, tag="x3pre", name="x3pre")
        for ob in range(WC):
            w2t = ffn.tile([128, MC, 128], BF16, tag="w2t", bufs=3,
                           name="w2t")
            nc.sync.dma_start(w2t[:], dram['ffn_w2t'][:, ob])
            ps = mm_ps.tile([128, T], F32, tag="mm", name="ps_f")
            for ic in range(MC):
                nc.tensor.matmul(ps[:, 0:T], w2t[:, ic, :], hT[:, ic, :],
                                 start=(ic == 0), stop=(ic == MC - 1))
            tmp = scratch.tile([128, T], F32, tag="ftmp", bufs=2,
                               name="f_tmp")
            nc.scalar.activation(tmp[:, 0:T], ps[:, 0:T], AF.Identity,
                                 bias=cols['b2'][:, ob:ob + 1])
            nc.vector.tensor_add(x3pre[:, ob, :], tmp[:, 0:T], x2T[:, ob, :])
        x3T = ffn.tile([128, WC, T], F32, tag="x3T", name="x3T")
        layernorm(lambda ic: x3pre[:, ic, :], WC, T,
                  cols['ln3_g'], cols['ln3_b'],
                  lambda ic: x3T[:, ic, :])
        # Output via PE transposes, g-major so each group pipelines right
        # behind its LN3 apply; per-tcx staging tiles let the 4 output DMAs
        # overlap the remaining transposes.
        o_tms = []
        for tcx in range(TC):
            o_tms.append(ffn.tile([128, W], F32, tag="o_tm", bufs=4,
                                  name="o_tm"))
        for tcx in range(TC):
            for g in range(WC // 4):
                pt = sc_ps.tile([128, 512], F32, tag="sc", name="pt_out")
                for k in range(4):
                    nc.tensor.transpose(
                        pt[:, k * 128:(k + 1) * 128],
                        x3T[:, g * 4 + k, tcx * 128:(tcx + 1) * 128],
                        ident[:])
                nc.vector.tensor_copy(
                    o_tms[tcx][:, g * 512:(g + 1) * 512], pt[:])
            nc.sync.dma_start(out_flat[tcx * 128:(tcx + 1) * 128, :],
                              o_tms[tcx][:])

    return out_d


_PROGRAM_CACHE = {}


def _get_program(B_loc, NQ, S, W, NH, MLP, JC=512, repeat=1):
    key = (B_loc, NQ, S, W, NH, MLP, JC, repeat)
    if key not in _PROGRAM_CACHE:
        nc = bacc.Bacc("TRN2", target_bir_lowering=False, debug=False)
        with tile.TileContext(nc) as tc, \
             nc.allow_low_precision(reason="bf16 matmul pipeline"):
            for r in range(repeat):
                with ExitStack() as ctx:
                    build_decoder(nc, tc, ctx, B_loc, NQ, S, W, NH, MLP, JC,
                                  suffix=("" if r == 0 else f"_r{r}"))
        nc.compile()
        _PROGRAM_CACHE[key] = nc
    return _PROGRAM_CACHE[key]


def _wt_blockmajor(w):
    """W [O, I] fp32 -> bf16 W^T block-major [128, O/128, I/128, 128]:
    wt[p, ob, c, n] = W[ob*128+n, c*128+p]."""
    O, I = w.shape
    # [ob, n, c, p] -> transpose to [p, ob, c, n]
    v = w.reshape(O // 128, 128, I // 128, 128).transpose(3, 0, 2, 1)
    return np.ascontiguousarray(v.astype(BF16_NP))


def _featmajor(x):
    """x [B, N, W] fp32 -> bf16 feature-major [128, W/128, B*N]:
    v[p, c, t] = x[b(t), n(t), c*128+p]."""
    B, N, W = x.shape
    v = x.reshape(B * N, W // 128, 128).transpose(2, 1, 0)
    return np.ascontiguousarray(v.astype(BF16_NP))


def _featmajor_fp8(x, scale):
    """Like _featmajor but scaled and cast to fp8e4m3."""
    B, N, W = x.shape
    v = (x * scale).reshape(B * N, W // 128, 128).transpose(2, 1, 0)
    return np.ascontiguousarray(v.astype(FP8_NP))


def _wt_dr(w, scale):
    """W [O, I] fp32 -> fp8 DoubleRow stationary [128, O/128, I/256, 2, 128]:
    v[p, ob, g, i, n] = W[ob*128+n, (2g+i)*128+p] * scale."""
    O, I = w.shape
    v = (w * scale).reshape(O // 128, 128, I // 256, 2, 128)
    v = v.transpose(4, 0, 2, 3, 1)
    return np.ascontiguousarray(v.astype(FP8_NP))


def _wt_mov(w, scale):
    """W [O, I] fp32 -> fp8 DoubleRow moving [128, I/256, 2, O]:
    v[p, g, i, f] = W[f, (2g+i)*128+p] * scale."""
    O, I = w.shape
    v = (w * scale).T.reshape(I // 256, 2, 128, O).transpose(2, 0, 1, 3)
    return np.ascontiguousarray(v.astype(FP8_NP))


def _col128(v):
    """[n] fp32 -> [128, n/128] per-partition columns: c[p, i] = v[i*128+p]."""
    return np.ascontiguousarray(v.reshape(-1, 128).T.astype(np.float32))


def _make_in_maps(inputs):
    B, NQ, W = inputs['query'].shape
    S = inputs['enc_mem'].shape[1]
    MLP = inputs['ffn_w1'].shape[0]
    B_loc = B // N_CORES
    f32 = {k: np.asarray(v, dtype=np.float32) for k, v in inputs.items()}

    shared = {}
    for p in ('sa', 'ca'):
        shared[f'{p}_wqt_dr'] = _wt_dr(f32[f'{p}_wq'], WSCALE)
        shared[f'{p}_wkt_dr'] = _wt_dr(f32[f'{p}_wk'], WSCALE)
        shared[f'{p}_wvt_mov'] = _wt_mov(f32[f'{p}_wv'], WSCALE)
        shared[f'{p}_wot_dr'] = _wt_dr(f32[f'{p}_wo'], WSCALE)
    shared['ffn_w1t'] = _wt_blockmajor(f32['ffn_w1'])
    shared['ffn_w2t'] = _wt_blockmajor(f32['ffn_w2'])
    for i in (1, 2, 3):
        for gb in ('g', 'b'):
            shared[f'ln{i}_{gb}_col'] = _col128(f32[f'ln{i}_{gb}'])
    shared['b1_col'] = _col128(f32['ffn_b1'])
    shared['b2_col'] = _col128(f32['ffn_b2'])

    q, pe, m = f32['query'], f32['out_pos_enc'], f32['enc_mem']
    qk = q + pe
    in_maps = []
    for c in range(N_CORES):
        sl = slice(c * B_loc, (c + 1) * B_loc)
        mp = dict(shared)
        mp['qT'] = _featmajor(q[sl])
        mp['qTq'] = _featmajor_fp8(q[sl], MSCALE)
        mp['qkTq'] = _featmajor_fp8(qk[sl], MSCALE)
        mp['peT16'] = _featmajor(pe[sl] * MSCALE)
        mp['mT'] = _featmajor_fp8(m[sl], MSCALE)
        in_maps.append(mp)
    return in_maps


def kernel(**inputs):
    B, NQ, W = inputs['query'].shape
    S = inputs['enc_mem'].shape[1]
    MLP = inputs['ffn_w1'].shape[0]
    NH = 16
    assert B % N_CORES == 0
    B_loc = B // N_CORES

    nc = _get_program(B_loc, NQ, S, W, NH, MLP)
    in_maps = _make_in_maps(inputs)

    res = run_bass_kernel_spmd(nc, in_maps, list(range(N_CORES)))
    return np.concatenate(
        [np.asarray(res.results[c]["out"]).astype(np.float32)
         for c in range(N_CORES)], axis=0)

